# revision 6
# baseline (speedup 1.0000x reference)
"""Trainium2 Bass kernel for nn_LogLinearAttention.

Math: the reference computes
    q = x@Wq.T+bq ; v = x@Wv.T+bv ; r = x@Wr.T+br
    scores = q @ v.T ; attn = softmax(scores, axis=1)   # over the QUERY axis
    emb[b,s,:] = sum_t attn[b,s,t] r[b,t,:] ; pooled = emb.sum(axis=1)
    out = sigmoid(pooled @ Wl.T + bl)

Because softmax normalizes over axis 1 and pooled sums over that same
axis, sum_s attn[s, t] == 1 for every t, so
    pooled[b] = sum_t r[b, t, :] = (sum_t x[b, t, :]) @ Wr.T + S*br
and the q/v projections and the S x S attention cancel exactly:
    out[b] = sigmoid( xsum[b] . (Wl@Wr) + S*(br . Wl) + bl )

The kernel therefore only needs a sequence-sum of x (the only large
input, 32MB total) plus tiny weight contractions. Data-parallel over
batch: core b handles x[b] (4MB), weights replicated.

Per-core device program (v2 — all fp32, HWDGE DMAs only):
  - x[b] arrives as [4, 128, 4, 512]: four 1MB sync-engine DMAs, each a
    contiguous slab with 8KB/partition runs (near peak HBM BW).
  - acc[128,512] += x chunk slices on the vector engine (15 adds,
    fully overlapped with the DMA stream).
  - w_rep[128,512] = broadcast(Wl@Wr) computed on the TensorEngine from
    one 1MB Wr DMA (scalar-engine HWDGE ring, concurrent with x) using a
    free-dim-broadcast stationary operand; runs mid-stream.
  - tail: acc *= w_rep ; row-reduce ; 128->1 matmul with ones ;
    sigmoid(+S*(br.Wl)+bl) on the scalar engine ; DMA the [1,1] out.
"""

import numpy as np

B, S, D = 8, 2048, 512
P = 128
NQ = 4  # x DMA chunks per core (1MB each)
KS = 4  # [128,512] slices per chunk
JW = 4  # Wr/Wl/br rows per partition

_CACHE = {}


def _build():
    import concourse.bacc as bacc
    import concourse.mybir as mybir
    import concourse.tile as tile

    f32 = mybir.dt.float32

    nc = bacc.Bacc(
        "TRN2",
        target_bir_lowering=False,
        debug=False,
        enable_asserts=False,
        num_devices=B,
    )
    x_d = nc.dram_tensor("x", [NQ, P, KS, D], f32, kind="ExternalInput").ap()
    wr_d = nc.dram_tensor("wr", [P, JW, D], f32, kind="ExternalInput").ap()
    wl_d = nc.dram_tensor("wl", [P, JW], f32, kind="ExternalInput").ap()
    br_d = nc.dram_tensor("br", [P, JW], f32, kind="ExternalInput").ap()
    bl_d = nc.dram_tensor("bl", [1, 1], f32, kind="ExternalInput").ap()
    out_d = nc.dram_tensor("out", [1, 1], f32, kind="ExternalOutput").ap()

    with tile.TileContext(nc) as tc:
        with (
            tc.tile_pool(name="xp", bufs=4) as xp,
            tc.tile_pool(name="sg", bufs=1) as sg,
            tc.tile_pool(name="ps", bufs=1, space="PSUM") as ps,
        ):
            ones = sg.tile([P, 1], f32, tag="ones")
            nc.vector.memset(ones, 1.0)
            wlt = sg.tile([P, JW], f32, tag="wlt")
            nc.scalar.dma_start(wlt, wl_d)
            brt = sg.tile([P, JW], f32, tag="brt")
            nc.scalar.dma_start(brt, br_d)
            blt = sg.tile([1, 1], f32, tag="blt")
            nc.scalar.dma_start(blt, bl_d)
            # Wr on the scalar-engine HWDGE ring so it streams
            # concurrently with the x chunks on the sync ring.
            wt = sg.tile([P, JW, D], f32, tag="wt")
            nc.scalar.dma_start(wt, wr_d)

            # constant term: t2 = S * dot(br, Wl) + bl   (off critical path)
            prod2 = sg.tile([P, JW], f32, tag="prod2")
            nc.vector.tensor_mul(out=prod2, in0=brt, in1=wlt)
            red2 = sg.tile([P, 1], f32, tag="red2")
            nc.vector.reduce_sum(red2, prod2, axis=mybir.AxisListType.X)
            c_ps = ps.tile([1, 1], f32, tag="c")
            nc.tensor.matmul(c_ps, red2, ones, start=True, stop=True)
            t2 = sg.tile([1, 1], f32, tag="t2")
            nc.scalar.activation(
                t2,
                c_ps,
                mybir.ActivationFunctionType.Identity,
                bias=blt,
                scale=float(S),
            )

            # w_rep[128, D]: every partition p holds w = Wl @ Wr.
            # lhsT[k, p] = Wl[4k+j] for all p via a free-dim broadcast of
            # the [128,1] Wl column; rhs = Wr rows in natural layout.
            wrep_ps = ps.tile([P, D], f32, tag="wrep")
            for j in range(JW):
                nc.tensor.matmul(
                    wrep_ps,
                    wlt[:, j : j + 1].to_broadcast([P, P]),
                    wt[:, j, :],
                    start=(j == 0),
                    stop=(j == JW - 1),
                )

            # acc[128, D] accumulates the x stream on the vector engine
            acc = sg.tile([P, D], f32, tag="acc")
            for q in range(NQ):
                xt = xp.tile([P, KS, D], f32, tag="xt")
                nc.sync.dma_start(xt, x_d[q])
                for k in range(KS):
                    if q == 0 and k == 0:
                        continue
                    if q == 0 and k == 1:
                        nc.vector.tensor_add(
                            out=acc, in0=xt[:, 0, :], in1=xt[:, 1, :]
                        )
                    else:
                        nc.vector.tensor_add(out=acc, in0=acc, in1=xt[:, k, :])

            # tail: logit = sum_{p,d} acc*w_rep + t2 ; sigmoid
            nc.vector.tensor_mul(out=acc, in0=acc, in1=wrep_ps)
            red = sg.tile([P, 1], f32, tag="red")
            nc.vector.reduce_sum(red, acc, axis=mybir.AxisListType.X)
            c2_ps = ps.tile([1, 1], f32, tag="c2")
            nc.tensor.matmul(c2_ps, red, ones, start=True, stop=True)
            fin = sg.tile([1, 1], f32, tag="fin")
            nc.scalar.activation(
                fin,
                c2_ps,
                mybir.ActivationFunctionType.Sigmoid,
                bias=t2,
                scale=1.0,
            )
            nc.sync.dma_start(out_d, fin)

    nc.compile()
    return nc


def _in_maps(inputs):
    x = np.ascontiguousarray(np.asarray(inputs["x"], dtype=np.float32))
    Wr = np.ascontiguousarray(np.asarray(inputs["Wr"], dtype=np.float32))
    br = np.asarray(inputs["br"], dtype=np.float32)
    Wl = np.asarray(inputs["Wl"], dtype=np.float32)
    bl = np.asarray(inputs["bl"], dtype=np.float32)

    wr_h = Wr.reshape(P, JW, D)
    wl_h = np.ascontiguousarray(Wl.reshape(P, JW))
    br_h = np.ascontiguousarray(br.reshape(P, JW))
    bl_h = bl.reshape(1, 1)
    return [
        {
            "x": x[b].reshape(NQ, P, KS, D),
            "wr": wr_h,
            "wl": wl_h,
            "br": br_h,
            "bl": bl_h,
        }
        for b in range(B)
    ]


def get_nc():
    if "nc" not in _CACHE:
        _CACHE["nc"] = _build()
    return _CACHE["nc"]


def kernel(**inputs) -> np.ndarray:
    from concourse.bass_utils import run_bass_kernel_spmd

    nc = get_nc()
    res = run_bass_kernel_spmd(nc, _in_maps(inputs), list(range(B)))
    out = np.stack([res.results[b]["out"].reshape(()) for b in range(B)])
    return out.reshape(B, 1).astype(np.float32)


# revision 7
# speedup vs baseline: 1.0215x; 1.0215x over previous
"""Trainium2 Bass kernel for nn_LogLinearAttention.

Math: the reference computes
    q = x@Wq.T+bq ; v = x@Wv.T+bv ; r = x@Wr.T+br
    scores = q @ v.T ; attn = softmax(scores, axis=1)   # over the QUERY axis
    emb[b,s,:] = sum_t attn[b,s,t] r[b,t,:] ; pooled = emb.sum(axis=1)
    out = sigmoid(pooled @ Wl.T + bl)

Because softmax normalizes over axis 1 and pooled sums over that same
axis, sum_s attn[s, t] == 1 for every t, so
    pooled[b] = sum_t r[b, t, :] = (sum_t x[b, t, :]) @ Wr.T + S*br
and the q/v projections and the S x S attention cancel exactly:
    out[b] = sigmoid( xsum[b] . (Wl@Wr) + S*(br . Wl) + bl )

The kernel therefore only needs a sequence-sum of x (the only large
input, 32MB total) plus tiny weight contractions. Data-parallel over
batch: core b handles x[b] (4MB), weights replicated.

Per-core device program (v3 — all fp32, HWDGE DMAs only):
  - x[b] arrives as [4, 128, 4, 512]: four 1MB sync-engine DMAs, each a
    contiguous slab with 8KB/partition runs.
  - All weights (Wr + Wl + br + bl) are packed host-side into ONE
    [128, 2057] tensor -> a single scalar-ring DMA. Every dma completion
    pays a ~3us straggler-engine lag here, so DMA count is minimized.
  - acc[128,512] += x chunk slices on the vector engine, FIRST in DVE
    program order so nothing blocks the chain.
  - w_rep[128,512] = broadcast(Wl@Wr) on the TensorEngine via a
    free-dim-broadcast stationary operand; runs mid-stream.
  - tail: acc *= w_rep ; row-reduce ; 128->1 matmul with ones ;
    sigmoid (table prewarmed at kernel start) ; DMA the [1,1] out.
"""

import numpy as np

B, S, D = 8, 2048, 512
P = 128
NQ = 4  # x DMA chunks per core (1MB each)
KS = 4  # [128,512] slices per chunk
JW = 4  # Wr/Wl/br rows per partition
WCOL = JW * D + 9  # packed weight columns: Wr | wl | br | bl pad

_CACHE = {}


def _build():
    import concourse.bacc as bacc
    import concourse.mybir as mybir
    import concourse.tile as tile

    f32 = mybir.dt.float32

    nc = bacc.Bacc(
        "TRN2",
        target_bir_lowering=False,
        debug=False,
        enable_asserts=False,
        num_devices=B,
    )
    x_d = nc.dram_tensor("x", [NQ, P, KS, D], f32, kind="ExternalInput").ap()
    wp_d = nc.dram_tensor("wp", [P, WCOL], f32, kind="ExternalInput").ap()
    out_d = nc.dram_tensor("out", [1, 1], f32, kind="ExternalOutput").ap()

    with tile.TileContext(nc) as tc:
        with (
            tc.tile_pool(name="xp", bufs=4) as xp,
            tc.tile_pool(name="sg", bufs=1) as sg,
            tc.tile_pool(name="ps", bufs=1, space="PSUM") as ps,
        ):
            ones = sg.tile([P, 1], f32, tag="ones")
            nc.vector.memset(ones, 1.0)
            # Prewarm the sigmoid activation table (1.3us) off the
            # critical path: a dummy [1,1] sigmoid right at the start.
            warm = sg.tile([1, 1], f32, tag="warm")
            nc.scalar.activation(
                warm, ones[0:1, 0:1], mybir.ActivationFunctionType.Sigmoid
            )

            # One DMA for every weight byte (scalar-engine HWDGE ring,
            # concurrent with the x stream on the sync ring).
            wp = sg.tile([P, WCOL], f32, tag="wp")
            nc.scalar.dma_start(wp, wp_d)
            wt = wp[:, : JW * D].rearrange("p (j d) -> p j d", j=JW)
            wlt = wp[:, JW * D : JW * D + JW]
            brt = wp[:, JW * D + JW : JW * D + 2 * JW]
            blt = wp[0:1, JW * D + 2 * JW : JW * D + 2 * JW + 1]

            # acc[128, D] accumulates the x stream on the vector engine.
            # Emitted FIRST in DVE program order so nothing blocks it.
            acc = sg.tile([P, D], f32, tag="acc")
            for q in range(NQ):
                xt = xp.tile([P, KS, D], f32, tag="xt")
                nc.sync.dma_start(xt, x_d[q])
                for k in range(KS):
                    if q == 0 and k == 0:
                        continue
                    if q == 0 and k == 1:
                        nc.vector.tensor_add(
                            out=acc, in0=xt[:, 0, :], in1=xt[:, 1, :]
                        )
                    else:
                        nc.vector.tensor_add(out=acc, in0=acc, in1=xt[:, k, :])

            # w_rep[128, D]: every partition p holds w = Wl @ Wr.
            # lhsT[k, p] = Wl[4k+j] for all p via a free-dim broadcast of
            # the [128,1] Wl column; rhs = Wr rows in natural layout.
            wrep_ps = ps.tile([P, D], f32, tag="wrep")
            for j in range(JW):
                nc.tensor.matmul(
                    wrep_ps,
                    wlt[:, j : j + 1].to_broadcast([P, P]),
                    wt[:, j, :],
                    start=(j == 0),
                    stop=(j == JW - 1),
                )

            # constant term: t2 = S * dot(br, Wl) + bl (off critical path,
            # DVE + one tiny matmul; no Identity activation table needed)
            prod2 = sg.tile([P, JW], f32, tag="prod2")
            nc.vector.tensor_mul(out=prod2, in0=brt, in1=wlt)
            red2 = sg.tile([P, 1], f32, tag="red2")
            nc.vector.reduce_sum(red2, prod2, axis=mybir.AxisListType.X)
            c_ps = ps.tile([1, 1], f32, tag="c")
            nc.tensor.matmul(c_ps, red2, ones, start=True, stop=True)
            t2 = sg.tile([1, 1], f32, tag="t2")
            nc.vector.tensor_scalar_mul(t2, c_ps, float(S))
            nc.vector.tensor_add(out=t2, in0=t2, in1=blt)

            # tail: logit = sum_{p,d} acc*w_rep + t2 ; sigmoid
            nc.vector.tensor_mul(out=acc, in0=acc, in1=wrep_ps)
            red = sg.tile([P, 1], f32, tag="red")
            nc.vector.reduce_sum(red, acc, axis=mybir.AxisListType.X)
            c2_ps = ps.tile([1, 1], f32, tag="c2")
            nc.tensor.matmul(c2_ps, red, ones, start=True, stop=True)
            fin = sg.tile([1, 1], f32, tag="fin")
            nc.scalar.activation(
                fin,
                c2_ps,
                mybir.ActivationFunctionType.Sigmoid,
                bias=t2,
                scale=1.0,
            )
            nc.sync.dma_start(out_d, fin)

    nc.compile()
    return nc


def _in_maps(inputs):
    x = np.ascontiguousarray(np.asarray(inputs["x"], dtype=np.float32))
    Wr = np.asarray(inputs["Wr"], dtype=np.float32)
    br = np.asarray(inputs["br"], dtype=np.float32)
    Wl = np.asarray(inputs["Wl"], dtype=np.float32)
    bl = np.asarray(inputs["bl"], dtype=np.float32)

    wp = np.zeros((P, WCOL), dtype=np.float32)
    wp[:, : JW * D] = Wr.reshape(P, JW * D)
    wp[:, JW * D : JW * D + JW] = Wl.reshape(P, JW)
    wp[:, JW * D + JW : JW * D + 2 * JW] = br.reshape(P, JW)
    wp[0, JW * D + 2 * JW] = bl[0]
    return [
        {"x": x[b].reshape(NQ, P, KS, D), "wp": wp}
        for b in range(B)
    ]


def get_nc():
    if "nc" not in _CACHE:
        _CACHE["nc"] = _build()
    return _CACHE["nc"]


def kernel(**inputs) -> np.ndarray:
    from concourse.bass_utils import run_bass_kernel_spmd

    nc = get_nc()
    res = run_bass_kernel_spmd(nc, _in_maps(inputs), list(range(B)))
    out = np.stack([res.results[b]["out"].reshape(()) for b in range(B)])
    return out.reshape(B, 1).astype(np.float32)


# revision 8
# speedup vs baseline: 1.1547x; 1.1304x over previous
"""Trainium2 Bass kernel for nn_LogLinearAttention.

Math: the reference computes
    q = x@Wq.T+bq ; v = x@Wv.T+bv ; r = x@Wr.T+br
    scores = q @ v.T ; attn = softmax(scores, axis=1)   # over the QUERY axis
    emb[b,s,:] = sum_t attn[b,s,t] r[b,t,:] ; pooled = emb.sum(axis=1)
    out = sigmoid(pooled @ Wl.T + bl)

Because softmax normalizes over axis 1 and pooled sums over that same
axis, sum_s attn[s, t] == 1 for every t, so
    pooled[b] = sum_t r[b, t, :] = (sum_t x[b, t, :]) @ Wr.T + S*br
and the q/v projections and the S x S attention cancel exactly:
    out[b] = sigmoid( xsum[b] . (Wl@Wr) + S*(br . Wl) + bl )

The kernel therefore only needs a sequence-sum of x (the only large
input, 32MB total) plus tiny weight contractions. Data-parallel over
batch: core b handles x[b] (4MB), weights replicated.

Per-core device program (v4 — all fp32, HWDGE DMAs only):
  - x[b] arrives as 16 slice DMAs of [128,512] (256KB each), split
    across the sync and scalar HWDGE rings, all issued up-front
    (bufs=16) so arrival is continuous from ~8us.
  - acc[128,512] += slice on the vector engine as each slice lands; the
    DVE stream carries NOTHING but these adds until the tail, so a
    late-arriving weight DMA can never stall the chain.
  - All weights (Wr+Wl+br+bl) pack into ONE [128,2057] DMA (every DMA
    completion pays a ~3us straggler-engine lag, so count is minimized).
  - w_rep[128,512] = broadcast(Wl@Wr) on the TensorEngine via a
    free-dim-broadcast stationary operand; runs mid-stream.
  - The bias constant S*(br.Wl)+bl is computed entirely on GpSimd
    (tensor ops + XYZWC reduce), keeping DVE and PE clear.
  - tail: acc *= w_rep ; row-reduce ; 128->1 matmul with ones ;
    sigmoid (table prewarmed at kernel start) ; DMA the [1,1] out.
"""

import numpy as np

B, S, D = 8, 2048, 512
P = 128
NSL = 16  # x slice DMAs per core (256KB each)
JW = 4  # Wr/Wl/br rows per partition
WCOL = JW * D + 9  # packed weight columns: Wr | wl | br | bl pad
N_SYNC = 12  # slices on the sync ring; rest go on the scalar ring

_CACHE = {}


def _build():
    import concourse.bacc as bacc
    import concourse.mybir as mybir
    import concourse.tile as tile

    f32 = mybir.dt.float32

    nc = bacc.Bacc(
        "TRN2",
        target_bir_lowering=False,
        debug=False,
        enable_asserts=False,
        num_devices=B,
    )
    x_d = nc.dram_tensor("x", [NSL, P, D], f32, kind="ExternalInput").ap()
    wp_d = nc.dram_tensor("wp", [P, WCOL], f32, kind="ExternalInput").ap()
    out_d = nc.dram_tensor("out", [1, 1], f32, kind="ExternalOutput").ap()

    with tile.TileContext(nc) as tc:
        with (
            tc.tile_pool(name="xp", bufs=NSL) as xp,
            tc.tile_pool(name="sg", bufs=1) as sg,
            tc.tile_pool(name="ps", bufs=1, space="PSUM") as ps,
        ):
            ones = sg.tile([P, 1], f32, tag="ones")
            nc.vector.memset(ones, 1.0)
            # Prewarm the sigmoid activation table (1.3us) off the
            # critical path: a dummy [1,1] sigmoid right at the start.
            warm = sg.tile([1, 1], f32, tag="warm")
            nc.scalar.activation(
                warm, ones[0:1, 0:1], mybir.ActivationFunctionType.Sigmoid
            )

            # One DMA for every weight byte, first on the scalar ring.
            wp = sg.tile([P, WCOL], f32, tag="wp")
            nc.scalar.dma_start(wp, wp_d)
            wt = wp[:, : JW * D].rearrange("p (j d) -> p j d", j=JW)
            wlt = wp[:, JW * D : JW * D + JW]
            brt = wp[:, JW * D + JW : JW * D + 2 * JW]
            blt = wp[0:1, JW * D + 2 * JW : JW * D + 2 * JW + 1]

            # acc[128, D] accumulates the x stream on the vector engine.
            # One DMA + one add per 256KB slice; nothing else ever enters
            # the DVE stream before the tail.
            acc = sg.tile([P, D], f32, tag="acc")
            xts = []
            for n in range(NSL):
                xt = xp.tile([P, D], f32, tag="xt")
                eng = nc.sync if n < N_SYNC else nc.scalar
                eng.dma_start(xt, x_d[n])
                xts.append(xt)
                if n == 1:
                    nc.vector.tensor_add(out=acc, in0=xts[0], in1=xts[1])
                elif n > 1:
                    nc.vector.tensor_add(out=acc, in0=acc, in1=xt)

            # w_rep[128, D]: every partition p holds w = Wl @ Wr.
            # lhsT[k, p] = Wl[4k+j] for all p via a free-dim broadcast of
            # the [128,1] Wl column; rhs = Wr rows in natural layout.
            wrep_ps = ps.tile([P, D], f32, tag="wrep")
            for j in range(JW):
                nc.tensor.matmul(
                    wrep_ps,
                    wlt[:, j : j + 1].to_broadcast([P, P]),
                    wt[:, j, :],
                    start=(j == 0),
                    stop=(j == JW - 1),
                )

            # constant term on GpSimd: t2 = S * dot(br, Wl) + bl
            prod2 = sg.tile([P, JW], f32, tag="prod2")
            nc.gpsimd.tensor_mul(out=prod2, in0=brt, in1=wlt)
            c_sb = sg.tile([1, 1], f32, tag="c_sb")
            nc.gpsimd.tensor_reduce(
                c_sb, prod2, axis=mybir.AxisListType.XYZWC, op=mybir.AluOpType.add
            )
            t2 = sg.tile([1, 1], f32, tag="t2")
            nc.gpsimd.tensor_scalar_mul(t2, c_sb, float(S))
            nc.gpsimd.tensor_add(out=t2, in0=t2, in1=blt)

            # tail: logit = sum_{p,d} acc*w_rep + t2 ; sigmoid
            nc.vector.tensor_mul(out=acc, in0=acc, in1=wrep_ps)
            red = sg.tile([P, 1], f32, tag="red")
            nc.vector.reduce_sum(red, acc, axis=mybir.AxisListType.X)
            c2_ps = ps.tile([1, 1], f32, tag="c2")
            nc.tensor.matmul(c2_ps, red, ones, start=True, stop=True)
            fin = sg.tile([1, 1], f32, tag="fin")
            nc.scalar.activation(
                fin,
                c2_ps,
                mybir.ActivationFunctionType.Sigmoid,
                bias=t2,
                scale=1.0,
            )
            nc.sync.dma_start(out_d, fin)

    nc.compile()
    return nc


def _in_maps(inputs):
    x = np.ascontiguousarray(np.asarray(inputs["x"], dtype=np.float32))
    Wr = np.asarray(inputs["Wr"], dtype=np.float32)
    br = np.asarray(inputs["br"], dtype=np.float32)
    Wl = np.asarray(inputs["Wl"], dtype=np.float32)
    bl = np.asarray(inputs["bl"], dtype=np.float32)

    wp = np.zeros((P, WCOL), dtype=np.float32)
    wp[:, : JW * D] = Wr.reshape(P, JW * D)
    wp[:, JW * D : JW * D + JW] = Wl.reshape(P, JW)
    wp[:, JW * D + JW : JW * D + 2 * JW] = br.reshape(P, JW)
    wp[0, JW * D + 2 * JW] = bl[0]
    return [
        {"x": x[b].reshape(NSL, P, D), "wp": wp}
        for b in range(B)
    ]


def get_nc():
    if "nc" not in _CACHE:
        _CACHE["nc"] = _build()
    return _CACHE["nc"]


def kernel(**inputs) -> np.ndarray:
    from concourse.bass_utils import run_bass_kernel_spmd

    nc = get_nc()
    res = run_bass_kernel_spmd(nc, _in_maps(inputs), list(range(B)))
    out = np.stack([res.results[b]["out"].reshape(()) for b in range(B)])
    return out.reshape(B, 1).astype(np.float32)


# revision 12
# speedup vs baseline: 1.1621x; 1.0064x over previous
"""Trainium2 Bass kernel for nn_LogLinearAttention.

Math: the reference computes
    q = x@Wq.T+bq ; v = x@Wv.T+bv ; r = x@Wr.T+br
    scores = q @ v.T ; attn = softmax(scores, axis=1)   # over the QUERY axis
    emb[b,s,:] = sum_t attn[b,s,t] r[b,t,:] ; pooled = emb.sum(axis=1)
    out = sigmoid(pooled @ Wl.T + bl)

Because softmax normalizes over axis 1 and pooled sums over that same
axis, sum_s attn[s, t] == 1 for every t, so
    pooled[b] = sum_t r[b, t, :] = (sum_t x[b, t, :]) @ Wr.T + S*br
and the q/v projections and the S x S attention cancel exactly:
    out[b] = sigmoid( xsum[b] . (Wl@Wr) + S*(br . Wl) + bl )

The kernel therefore only needs a sequence-sum of x (the only large
input, 32MB total) plus tiny weight contractions. Data-parallel over
batch: core b handles x[b] (4MB), weights replicated.

Per-core device program (v4 — all fp32, HWDGE DMAs only):
  - x[b] arrives as 16 slice DMAs of [128,512] (256KB each), split
    across the sync and scalar HWDGE rings, all issued up-front
    (bufs=16) so arrival is continuous from ~8us.
  - acc[128,512] += slice on the vector engine as each slice lands; the
    DVE stream carries NOTHING but these adds until the tail, so a
    late-arriving weight DMA can never stall the chain.
  - All weights (Wr+Wl+br+bl) pack into ONE [128,2057] DMA (every DMA
    completion pays a ~3us straggler-engine lag, so count is minimized).
  - w_rep[128,512] = broadcast(Wl@Wr) on the TensorEngine via a
    free-dim-broadcast stationary operand; runs mid-stream.
  - The bias constant S*(br.Wl)+bl is computed entirely on GpSimd
    (tensor ops + XYZWC reduce), keeping DVE and PE clear.
  - tail: acc *= w_rep ; row-reduce ; 128->1 matmul with ones ;
    sigmoid (table prewarmed at kernel start) ; DMA the [1,1] out.
"""

import numpy as np

B, S, D = 8, 2048, 512
P = 128
NSL = 16  # x slice DMAs per core (256KB each)
JW = 4  # Wr/Wl/br rows per partition
WCOL = JW * D + 9  # packed weight columns: Wr | wl | br | bl pad
N_SYNC = 10  # slices on the sync ring; rest go on the scalar ring
# ring loads: sync 10 x 256KB = 2.5MB ; scalar = wp (1MB) + 6 x 256KB = 2.5MB

_CACHE = {}


def _build():
    import concourse.bacc as bacc
    import concourse.mybir as mybir
    import concourse.tile as tile

    f32 = mybir.dt.float32

    nc = bacc.Bacc(
        "TRN2",
        target_bir_lowering=False,
        debug=False,
        enable_asserts=False,
        num_devices=B,
    )
    x_d = nc.dram_tensor("x", [NSL, P, D], f32, kind="ExternalInput").ap()
    wp_d = nc.dram_tensor("wp", [P, WCOL], f32, kind="ExternalInput").ap()
    out_d = nc.dram_tensor("out", [1, 1], f32, kind="ExternalOutput").ap()

    with tile.TileContext(nc) as tc:
        with (
            tc.tile_pool(name="xp", bufs=NSL) as xp,
            tc.tile_pool(name="sg", bufs=1) as sg,
            tc.tile_pool(name="ps", bufs=1, space="PSUM") as ps,
        ):
            ones = sg.tile([P, 1], f32, tag="ones")
            nc.vector.memset(ones, 1.0)
            # Prewarm the sigmoid activation table (1.3us) off the
            # critical path: a dummy [1,1] sigmoid right at the start.
            warm = sg.tile([1, 1], f32, tag="warm")
            nc.scalar.activation(
                warm, ones[0:1, 0:1], mybir.ActivationFunctionType.Sigmoid
            )

            # One DMA for every weight byte, first on the scalar ring.
            wp = sg.tile([P, WCOL], f32, tag="wp")
            nc.scalar.dma_start(wp, wp_d)
            wt = wp[:, : JW * D].rearrange("p (j d) -> p j d", j=JW)
            wlt = wp[:, JW * D : JW * D + JW]
            brt = wp[:, JW * D + JW : JW * D + 2 * JW]
            blt = wp[0:1, JW * D + 2 * JW : JW * D + 2 * JW + 1]

            # acc[128, D] accumulates the x stream on the vector engine.
            # One DMA + one add per 256KB slice; nothing else ever enters
            # the DVE stream before the tail.
            acc = sg.tile([P, D], f32, tag="acc")
            xts = []
            for n in range(NSL):
                xt = xp.tile([P, D], f32, tag="xt")
                eng = nc.sync if n < N_SYNC else nc.scalar
                eng.dma_start(xt, x_d[n])
                xts.append(xt)
                if n == 1:
                    nc.vector.tensor_add(out=acc, in0=xts[0], in1=xts[1])
                elif n > 1:
                    nc.vector.tensor_add(out=acc, in0=acc, in1=xt)

            # w_rep[128, D]: every partition p holds w = Wl @ Wr.
            # lhsT[k, p] = Wl[4k+j] for all p via a free-dim broadcast of
            # the [128,1] Wl column; rhs = Wr rows in natural layout.
            wrep_ps = ps.tile([P, D], f32, tag="wrep")
            for j in range(JW):
                nc.tensor.matmul(
                    wrep_ps,
                    wlt[:, j : j + 1].to_broadcast([P, P]),
                    wt[:, j, :],
                    start=(j == 0),
                    stop=(j == JW - 1),
                )

            # constant term on GpSimd: t2 = S * dot(br, Wl) + bl
            prod2 = sg.tile([P, JW], f32, tag="prod2")
            nc.gpsimd.tensor_mul(out=prod2, in0=brt, in1=wlt)
            c_sb = sg.tile([1, 1], f32, tag="c_sb")
            nc.gpsimd.tensor_reduce(
                c_sb, prod2, axis=mybir.AxisListType.XYZWC, op=mybir.AluOpType.add
            )
            t2 = sg.tile([1, 1], f32, tag="t2")
            nc.gpsimd.tensor_scalar_mul(t2, c_sb, float(S))
            nc.gpsimd.tensor_add(out=t2, in0=t2, in1=blt)

            # tail: logit = sum_{p,d} acc*w_rep + t2 ; sigmoid.
            # (tensor_tensor_reduce would fuse these two DVE passes but
            # crashes the NEFF at execute time on this toolchain.)
            nc.vector.tensor_mul(out=acc, in0=acc, in1=wrep_ps)
            red = sg.tile([P, 1], f32, tag="red")
            nc.vector.reduce_sum(red, acc, axis=mybir.AxisListType.X)
            c2_ps = ps.tile([1, 1], f32, tag="c2")
            nc.tensor.matmul(c2_ps, red, ones, start=True, stop=True)
            fin = sg.tile([1, 1], f32, tag="fin")
            nc.scalar.activation(
                fin,
                c2_ps,
                mybir.ActivationFunctionType.Sigmoid,
                bias=t2,
                scale=1.0,
            )
            # out goes on the scalar ring: the sync ring is still
            # retiring the last x slices when fin is ready.
            nc.scalar.dma_start(out_d, fin)

    nc.compile()
    return nc


def _in_maps(inputs):
    x = np.ascontiguousarray(np.asarray(inputs["x"], dtype=np.float32))
    Wr = np.asarray(inputs["Wr"], dtype=np.float32)
    br = np.asarray(inputs["br"], dtype=np.float32)
    Wl = np.asarray(inputs["Wl"], dtype=np.float32)
    bl = np.asarray(inputs["bl"], dtype=np.float32)

    wp = np.zeros((P, WCOL), dtype=np.float32)
    wp[:, : JW * D] = Wr.reshape(P, JW * D)
    wp[:, JW * D : JW * D + JW] = Wl.reshape(P, JW)
    wp[:, JW * D + JW : JW * D + 2 * JW] = br.reshape(P, JW)
    wp[0, JW * D + 2 * JW] = bl[0]
    return [
        {"x": x[b].reshape(NSL, P, D), "wp": wp}
        for b in range(B)
    ]


def get_nc():
    if "nc" not in _CACHE:
        _CACHE["nc"] = _build()
    return _CACHE["nc"]


def kernel(**inputs) -> np.ndarray:
    from concourse.bass_utils import run_bass_kernel_spmd

    nc = get_nc()
    res = run_bass_kernel_spmd(nc, _in_maps(inputs), list(range(B)))
    out = np.stack([res.results[b]["out"].reshape(()) for b in range(B)])
    return out.reshape(B, 1).astype(np.float32)
